# revision 1
# baseline (speedup 1.0000x reference)
"""Trainium2 Bass kernel for nn_Decoder (6-layer transformer decoder).

Strategy: data-parallel over batch B=16 across 8 NeuronCores (2 sequences
per core), weights replicated. Per core everything is computed feature-major
(activations stored transposed, [features on partitions, tokens on free dim])
so every linear layer is a weight-stationary fp32r matmul and no on-device
transposes are needed:

  - projections:   out^T = W.T @ x^T        (W is already [d_in, d_out])
  - scores:        S^T[t,q] = K_h Q_h^T     (K=64 contraction, heads packed
                                             two-per-PE-pass at partition 0/64)
  - softmax:       P = exp(S^T/8 + causal mask); denominator comes for free
                   from an all-ones column appended to token-major V
  - AV:            O^T = [V;1]^T P  -> [65, 512] PSUM, row 64 = denominator
  - layernorm:     token-wise stats via all-ones stationary matmuls
                   (replicated over partitions), rstd = exp(-0.5*ln(var+eps))
                   so ACT stays on the natural_log_exp table set

Host side transposes x/y/weights into these layouts (numpy), shards the
batch, and runs the single compiled Bass program SPMD on cores 0-7.
"""
import sys

if "/opt/trn_rl_repo" not in sys.path:
    sys.path.insert(0, "/opt/trn_rl_repo")

import ml_dtypes
import numpy as np

import concourse.bass as bass
import concourse.mybir as mybir
import concourse.tile as tile
from concourse import bacc
from concourse.bass_utils import run_bass_kernel_spmd

# The ACT-table placement pass maps Exp -> "exp_and_others" and Ln ->
# "natural_log", so a kernel using both thrashes ACT_TABLE_LOADs (~1.3us
# each) inside the softmax/LN chain.  Advertise Exp/Ln only from the
# combined "natural_log_exp_and_others" set (indices are preserved, so the
# emitted act_func_set_id still matches act_info.json) -> one load total.
_orig_get_act_tables = bacc.get_activation_tables


def _patched_get_act_tables(arch):
    tables = dict(_orig_get_act_tables(arch))
    exp = mybir.ActivationFunctionType.Exp
    ln = mybir.ActivationFunctionType.Ln
    if any(exp in f and ln in f for f in tables.values()):
        out = {}
        for name, fns in tables.items():
            if exp in fns and ln in fns:
                out[name] = fns
            else:
                out[name] = fns - {exp, ln}
        return out
    return tables


bacc.get_activation_tables = _patched_get_act_tables

_bf16 = ml_dtypes.bfloat16
F32R = mybir.dt.float32r
F32 = mybir.dt.float32
BF16 = mybir.dt.bfloat16
AF = mybir.ActivationFunctionType
ALU = mybir.AluOpType

L, H, D, DH, DFF = 6, 8, 512, 64, 2048
EPS = 1e-5
NCORES = 8
BLOC = 2            # sequences per core
S = 512             # tokens per sequence
TLOC = BLOC * S     # tokens per core
KT = D // 128       # 4 contraction k-tiles for D
MT = D // 128       # 4 output feature m-tiles
NH = 4              # FFN token chunks (256 tokens each)
FCH = TLOC // NH    # 256


def _build(trivial_ln: bool, trivial_bias: bool):
    nc = bacc.Bacc("TRN2", target_bir_lowering=False, debug=False)

    xT = nc.dram_tensor("xT", [D, TLOC], F32R, kind="ExternalInput")
    yT = nc.dram_tensor("yT", [D, TLOC], F32R, kind="ExternalInput")
    wq1 = nc.dram_tensor("wq1", [L, D, D], F32R, kind="ExternalInput")
    wk1 = nc.dram_tensor("wk1", [L, D, D], F32R, kind="ExternalInput")
    wv1 = nc.dram_tensor("wv1", [L, D, D], F32R, kind="ExternalInput")
    wo1 = nc.dram_tensor("wo1", [L, D, D], F32R, kind="ExternalInput")
    wq2 = nc.dram_tensor("wq2", [L, D, D], F32R, kind="ExternalInput")
    wk2 = nc.dram_tensor("wk2", [L, D, D], F32R, kind="ExternalInput")
    wv2 = nc.dram_tensor("wv2", [L, D, D], F32R, kind="ExternalInput")
    wo2 = nc.dram_tensor("wo2", [L, D, D], F32R, kind="ExternalInput")
    w1 = nc.dram_tensor("w1", [L, D, DFF], BF16, kind="ExternalInput")
    w2 = nc.dram_tensor("w2", [L, DFF, D], BF16, kind="ExternalInput")
    if not trivial_ln:
        lng = nc.dram_tensor("lng", [3, L, D], F32, kind="ExternalInput")
        lnb = nc.dram_tensor("lnb", [3, L, D], F32, kind="ExternalInput")
    if not trivial_bias:
        bf1 = nc.dram_tensor("bf1", [L, DFF], F32, kind="ExternalInput")
        bf2 = nc.dram_tensor("bf2", [L, D], F32, kind="ExternalInput")
    outT = nc.dram_tensor("outT", [D, TLOC], F32R, kind="ExternalOutput")

    from contextlib import ExitStack
    with ExitStack() as _ctx:
        tc = _ctx.enter_context(tile.TileContext(nc))
        _ctx.enter_context(nc.allow_low_precision(reason="fp32r matmul inputs"))

        def _pool(name, bufs, space="SBUF"):
            return _ctx.enter_context(tc.tile_pool(name=name, bufs=bufs, space=space))

        cst = _pool("cst", 1)
        hidp = _pool("hid", 6)
        sresp = _pool("sres", 4)
        sqp = _pool("sq", 1)
        qkp = _pool("qk", 4)
        vvp = _pool("vv", 4)
        ptp = _pool("pt", 3)
        denp = _pool("den", 1)
        recp = _pool("rec", 2)
        osbp = _pool("osb", 3)
        oalp = _pool("oal", 4)
        f1p = _pool("f1", 16)
        awqp = _pool("awq", 4)
        awkp = _pool("awk", 4)
        awvp = _pool("awv", 4)
        awop = _pool("awo", 4)
        w1p = _pool("w1p", 4)
        w2p = _pool("w2p", 16)
        h2bp = _pool("h2b", 4)
        ybp = _pool("ybp", 4)
        stp = _pool("st", 4)
        t1p = _pool("t1", 3)
        b1p = _pool("b1", 16)
        tinyp = _pool("tiny", 4)
        psA = _pool("psA", 4, "PSUM")
        psB = _pool("psB", 3, "PSUM")
        psC = _pool("psC", 1, "PSUM")

        if True:
            # ---------------- constants ----------------
            onesf = cst.tile([128, 128], F32, tag="onesf", name="onesf")
            nc.gpsimd.memset(onesf[:], 1.0)
            ones = cst.tile([128, 128], F32R, tag="ones", name="ones")
            nc.vector.tensor_copy(ones[:], onesf[:])
            # causal diag-block mask: keep (0) where t(p) <= q(j), else -1e30
            maskD = cst.tile([128, 128], F32, tag="maskD")
            nc.gpsimd.memset(maskD[:], 0.0)
            nc.gpsimd.affine_select(
                out=maskD[:], in_=maskD[:], compare_op=ALU.is_ge,
                fill=-1e30, base=0, pattern=[[1, 128]], channel_multiplier=-1,
            )
            epsb = cst.tile([128, 1], F32, tag="epsb", name="epsb")
            nc.gpsimd.memset(epsb[:], EPS)

            # ---------------- initial activation load ----------------
            xcur = []
            for k in range(KT):
                t = hidp.tile([128, TLOC], F32R, tag="hid", name="hid")
                nc.sync.dma_start(t[:], xT.ap()[k * 128:(k + 1) * 128, :])
                xcur.append(t)

            # persistent token-major V tiles (8 = 4 t-tiles x 2 seqs), with
            # the all-ones denominator column written once
            vper = []
            for i in range(4):
                va = vvp.tile([128, H * 65], F32R, tag="vv", name="vv")
                nc.vector.tensor_copy(
                    va[:].rearrange("p (h c) -> p h c", h=H)[:, :, 64:65],
                    onesf[:, 0:H].rearrange("p (h o) -> p h o", o=1))
                vper.append(va)

            def load_proj_w(pool, dram, l, tag):
                tiles = []
                for k in range(KT):
                    t = pool.tile([128, D], F32R, tag=tag)
                    nc.sync.dma_start(t[:], dram.ap()[l, k * 128:(k + 1) * 128, :])
                    tiles.append(t)
                return tiles

            def ln_params(idx, l):
                """per-feature gain/bias columns for LN idx (0..2) of layer l."""
                if trivial_ln:
                    return None, None
                gs, bs = [], []
                for k in range(KT):
                    g = tinyp.tile([128, 1], F32, tag="lng", name="lng")
                    nc.sync.dma_start(
                        g[:], lng.ap()[idx, l, k * 128:(k + 1) * 128].rearrange("p -> p 1"))
                    b = tinyp.tile([128, 1], F32, tag="lnb", name="lnb")
                    nc.sync.dma_start(
                        b[:], lnb.ap()[idx, l, k * 128:(k + 1) * 128].rearrange("p -> p 1"))
                    gs.append(g)
                    bs.append(b)
                return gs, bs

            def layer_norm(src, idx, l, last=False, bf16_pool=None):
                """src: 4 tiles [128, TLOC] fp32r. Returns 4 new hid tiles
                (plus parallel bf16 copies when bf16_pool is given)."""
                gs, bs = ln_params(idx, l)
                outb = ([bf16_pool.tile([128, TLOC], BF16, tag="h2b", name="h2b")
                         for _ in range(KT)] if bf16_pool else None)
                sq = []
                for k in range(KT):
                    s = sqp.tile([128, TLOC], F32R, tag="sq", name="sq")
                    nc.gpsimd.tensor_mul(s[:], src[k][:], src[k][:])
                    sq.append(s)
                out = [hidp.tile([128, TLOC], F32R, tag="hid", name="hid") for _ in range(KT)]
                for nh in range(2):
                    cs = slice(nh * 512, (nh + 1) * 512)
                    sum_ps = psB.tile([128, 512], F32, tag="sc", name="sc")
                    ssq_ps = psB.tile([128, 512], F32, tag="sc", name="sc")
                    for k in range(KT):
                        nc.tensor.matmul(sum_ps[:], ones[:], src[k][:, cs],
                                         start=(k == 0), stop=(k == KT - 1))
                    for k in range(KT):
                        nc.tensor.matmul(ssq_ps[:], ones[:], sq[k][:, cs],
                                         start=(k == 0), stop=(k == KT - 1))
                    m = stp.tile([128, 512], F32, tag="st", name="st")
                    nc.vector.tensor_scalar_mul(m[:], sum_ps[:], 1.0 / D)
                    t1s = []
                    for k in range(KT):
                        t1 = t1p.tile([128, 512], F32, tag="t1", name="t1")
                        nc.vector.tensor_sub(t1[:], src[k][:, cs], m[:])
                        t1s.append(t1)
                    msq = stp.tile([128, 512], F32, tag="st", name="st")
                    nc.vector.tensor_mul(msq[:], m[:], m[:])
                    var = stp.tile([128, 512], F32, tag="st", name="st")
                    nc.vector.scalar_tensor_tensor(
                        var[:], ssq_ps[:], 1.0 / D, msq[:], ALU.mult, ALU.subtract)
                    lnv = stp.tile([128, 512], F32, tag="st", name="st")
                    nc.scalar.activation(lnv[:], var[:], AF.Ln, bias=epsb[:])
                    rstd = stp.tile([128, 512], F32, tag="st", name="st")
                    nc.scalar.activation(rstd[:], lnv[:], AF.Exp, scale=-0.5)
                    for k in range(KT):
                        if trivial_ln:
                            if outb is not None:
                                # bf16 copy first: the FFN only needs this one,
                                # so it must not queue behind the fp32r write
                                nc.vector.tensor_mul(
                                    outb[k][:, cs], t1s[k][:], rstd[:])
                            nc.vector.tensor_mul(out[k][:, cs], t1s[k][:], rstd[:])
                        else:
                            t2 = t1p.tile([128, 512], F32, tag="t2", name="t2")
                            nc.vector.tensor_mul(t2[:], t1s[k][:], rstd[:])
                            nc.vector.tensor_scalar(
                                out[k][:, cs], t2[:], gs[k][:], bs[k][:],
                                ALU.mult, ALU.add)
                            if outb is not None:
                                nc.vector.tensor_copy(outb[k][:, cs], out[k][:, cs])
                        if last:
                            nc.sync.dma_start(
                                outT.ap()[k * 128:(k + 1) * 128, cs], out[k][:, cs])
                return (out, outb) if bf16_pool else out

            def attention(qsrc, kv_from_y, l, wq_d, wk_d, wv_d, wo_d, causal, resid_src):
                """Full MHA block. qsrc: 4 fm tiles (queries). K/V from y (cross)
                or qsrc (self). Returns s = attn_out + resid (4 sres tiles)."""
                wq_t = load_proj_w(awqp, wq_d, l, "awq")
                wk_t = load_proj_w(awkp, wk_d, l, "awk")
                wv_t = load_proj_w(awvp, wv_d, l, "awv")
                wo_t = load_proj_w(awop, wo_d, l, "awo")
                oall = [oalp.tile([128, TLOC], F32R, tag="oal", name="oal") for _ in range(MT)]
                for b in range(BLOC):
                    bs = slice(b * S, (b + 1) * S)
                    if kv_from_y:
                        kvsrc = []
                        for k in range(KT):
                            t = ybp.tile([128, S], F32R, tag="yb", name="yb")
                            nc.sync.dma_start(
                                t[:], yT.ap()[k * 128:(k + 1) * 128, bs])
                            kvsrc.append(t)
                        kvs = [(t, slice(0, S)) for t in kvsrc]
                    else:
                        kvs = [(qsrc[k], bs) for k in range(KT)]
                    # Q/K projections, feature-major [D, S]
                    qb, kb = [], []
                    for (dst, wt, src_tiles, tg) in (
                            (qb, wq_t, [(qsrc[k], bs) for k in range(KT)], "q"),
                            (kb, wk_t, kvs, "k")):
                        for mi in range(MT):
                            ps = psA.tile([128, 512], F32, tag="mm", name="mm")
                            for k in range(KT):
                                st_, sl_ = src_tiles[k]
                                nc.tensor.matmul(
                                    ps[:], wt[k][:, mi * 128:(mi + 1) * 128],
                                    st_[:, sl_], start=(k == 0), stop=(k == KT - 1))
                            o = qkp.tile([128, S], F32R, tag=tg, name=tg)
                            if tg == "k":
                                nc.scalar.copy(o[:], ps[:])
                            else:
                                nc.vector.tensor_copy(o[:], ps[:])
                            dst.append(o)
                    # V token-major augmented: [128 tok, 8*(64+1)]
                    vb = []
                    for tt in range(4):
                        ps = psA.tile([128, 512], F32, tag="mm", name="mm")
                        for k in range(KT):
                            st_, sl_ = kvs[k]
                            t0 = sl_.start + tt * 128
                            nc.tensor.matmul(
                                ps[:], st_[:, t0:t0 + 128], wv_t[k][:],
                                start=(k == 0), stop=(k == KT - 1))
                        va = vper[tt]
                        nc.vector.tensor_copy(
                            va[:].rearrange("p (h c) -> p h c", h=H)[:, :, 0:64],
                            ps[:].rearrange("p (h c) -> p h c", h=H))
                        vb.append(va)
                    # heads, in pairs: softmax denominators for both heads
                    # land in one [33, 512] tile (rows 0 / 32) and get one
                    # batched reciprocal on the scalar engine as exp(-ln(x)).
                    # The normalize (bc matmul + mul) runs one group behind so
                    # the in-order PE stream never waits on the ACT chain.
                    pending = []

                    def flush_norm(item):
                        phg, prec, posbu = item
                        for hh in range(2):
                            h = phg * 2 + hh
                            hb = (h % 2) * 64
                            mi = h // 2
                            r = hh * 32
                            bc = psB.tile([64, 512], F32, tag="sc", name="sc")
                            nc.tensor.matmul(
                                bc[:], ones[r:r + 1, 0:64], prec[r:r + 1, :],
                                start=True, stop=True)
                            nc.vector.tensor_mul(
                                oall[mi][hb:hb + 64, bs], bc[:], posbu[hh][:])

                    for hg in range(H // 2):
                        if len(pending) > 1:
                            flush_norm(pending.pop(0))
                        den = denp.tile([33, 512], F32, tag="den", name="den")
                        osbu = []
                        for hh in range(2):
                            h = hg * 2 + hh
                            hb = (h % 2) * 64
                            mi = h // 2
                            pts = []
                            for tt in range(4):
                                n0 = tt * 128 if causal else 0
                                sc_ps = psB.tile([128, 512], F32, tag="sc", name="sc")
                                nc.tensor.matmul(
                                    sc_ps[:, n0:512],
                                    kb[mi][hb:hb + 64, tt * 128:(tt + 1) * 128],
                                    qb[mi][hb:hb + 64, n0:512],
                                    start=True, stop=True)
                                ptt = ptp.tile([128, 512], F32R, tag="pt", name="pt")
                                if causal:
                                    nc.vector.tensor_add(
                                        sc_ps[:, n0:n0 + 128], sc_ps[:, n0:n0 + 128],
                                        maskD[:])
                                nc.scalar.activation(
                                    ptt[:, n0:512], sc_ps[:, n0:512], AF.Exp,
                                    scale=0.125)
                                pts.append(ptt)
                            av = psC.tile([65, 512], F32, tag="av", name="av")
                            for tt in range(4):
                                n0 = tt * 128 if causal else 0
                                nc.tensor.matmul(
                                    av[:, n0:512],
                                    vb[tt][:, h * 65:h * 65 + 65],
                                    pts[tt][:, n0:512],
                                    start=(tt == 0), stop=(tt == 3))
                            nc.vector.tensor_copy(
                                den[hh * 32:hh * 32 + 1, :], av[64:65, :])
                            ou = osbp.tile([64, 512], F32, tag="osb", name="osb")
                            nc.vector.tensor_copy(ou[:], av[0:64, :])
                            osbu.append(ou)
                        lnd = denp.tile([33, 512], F32, tag="lnd", name="lnd")
                        nc.scalar.activation(lnd[:], den[:], AF.Ln)
                        rec = recp.tile([33, 512], F32R, tag="rec", name="rec")
                        nc.scalar.activation(rec[:], lnd[:], AF.Exp, scale=-1.0)
                        pending.append((hg, rec, osbu))
                    while pending:
                        flush_norm(pending.pop(0))
                # output projection + residual
                s_out = [sresp.tile([128, TLOC], F32R, tag="sres", name="sres") for _ in range(MT)]
                for mi in range(MT):
                    for nh in range(2):
                        cs = slice(nh * 512, (nh + 1) * 512)
                        ps = psA.tile([128, 512], F32, tag="mm", name="mm")
                        for k in range(KT):
                            nc.tensor.matmul(
                                ps[:], wo_t[k][:, mi * 128:(mi + 1) * 128],
                                oall[k][:, cs], start=(k == 0), stop=(k == KT - 1))
                        nc.vector.tensor_add(
                            s_out[mi][:, cs], ps[:], resid_src[mi][:, cs])
                return s_out

            # ================= layer loop =================
            for l in range(L):
                # FFN weights for this layer: issued up front on the (idle)
                # GPSIMD SWDGE queues so they never contend with the SP HWDGE
                # stream that feeds attention weights.
                w1_t = []
                for k in range(KT):
                    t = w1p.tile([128, DFF], BF16, tag="w1", name="w1")
                    nc.gpsimd.dma_start(
                        t[:], w1.ap()[l, k * 128:(k + 1) * 128, :])
                    w1_t.append(t)
                w2_t = []
                for k in range(DFF // 128):
                    t = w2p.tile([128, D], BF16, tag="w2", name="w2")
                    nc.gpsimd.dma_start(
                        t[:], w2.ap()[l, k * 128:(k + 1) * 128, :])
                    w2_t.append(t)
                # ---- masked self-attention + LN1 ----
                s1 = attention(xcur, False, l, wq1, wk1, wv1, wo1, True, xcur)
                h1 = layer_norm(s1, 0, l)
                # ---- cross-attention + LN2 ----
                s2 = attention(h1, True, l, wq2, wk2, wv2, wo2, False, h1)
                h2, h2b = layer_norm(s2, 1, l, bf16_pool=h2bp)
                # ---- FFN ----
                if not trivial_bias:
                    b1c, b2c = [], []
                    for i in range(DFF // 128):
                        t = b1p.tile([128, 1], F32, tag="b1c", name="b1c")
                        nc.sync.dma_start(
                            t[:], bf1.ap()[l, i * 128:(i + 1) * 128].rearrange("p -> p 1"))
                        b1c.append(t)
                    for i in range(MT):
                        t = tinyp.tile([128, 1], F32, tag="b2c", name="b2c")
                        nc.sync.dma_start(
                            t[:], bf2.ap()[l, i * 128:(i + 1) * 128].rearrange("p -> p 1"))
                        b2c.append(t)
                s3 = [sresp.tile([128, TLOC], F32R, tag="sres", name="sres") for _ in range(MT)]
                for nh in range(NH):
                    cs = slice(nh * FCH, (nh + 1) * FCH)
                    f1t = []
                    for m in range(DFF // 128):
                        ps = psA.tile([128, FCH], F32, tag="mm", name="mm")
                        for k in range(KT):
                            nc.tensor.matmul(
                                ps[:], w1_t[k][:, m * 128:(m + 1) * 128],
                                h2b[k][:, cs], start=(k == 0), stop=(k == KT - 1))
                        f = f1p.tile([128, FCH], BF16, tag="f1", name="f1")
                        nc.scalar.activation(
                            f[:], ps[:], AF.Relu,
                            bias=0.0 if trivial_bias else b1c[m][:])
                        f1t.append(f)
                    # FFN2: s3 = f1 @ W2 + bf2 + h2
                    fps = [psA.tile([128, FCH], F32, tag="mm", name="mm") for _ in range(MT)]
                    for k in range(DFF // 128):
                        for m in range(MT):
                            nc.tensor.matmul(
                                fps[m][:], w2_t[k][:, m * 128:(m + 1) * 128],
                                f1t[k][:], start=(k == 0), stop=(k == DFF // 128 - 1))
                    for m in range(MT):
                        nc.vector.scalar_tensor_tensor(
                            s3[m][:, cs], fps[m][:],
                            0.0 if trivial_bias else b2c[m][:],
                            h2[m][:, cs], ALU.add, ALU.add)
                xcur = layer_norm(s3, 2, l, last=(l == L - 1))

    nc.compile()
    return nc


_NC_CACHE = {}


def _get_nc(trivial_ln, trivial_bias):
    key = (trivial_ln, trivial_bias)
    if key not in _NC_CACHE:
        _NC_CACHE[key] = _build(trivial_ln, trivial_bias)
    return _NC_CACHE[key]


def _prep_inputs(inputs):
    f = np.float32
    x = np.asarray(inputs["x"], f)
    y = np.asarray(inputs["y"], f)

    def fm(w):  # [L, H, D, DH] -> [L, D, H*DH]
        return np.ascontiguousarray(
            np.asarray(w, f).transpose(0, 2, 1, 3).reshape(L, D, H * DH))

    shared = {
        "wq1": fm(inputs["Wq1"]), "wk1": fm(inputs["Wk1"]), "wv1": fm(inputs["Wv1"]),
        "wo1": np.ascontiguousarray(np.asarray(inputs["Wo1"], f)),
        "wq2": fm(inputs["Wq2"]), "wk2": fm(inputs["Wk2"]), "wv2": fm(inputs["Wv2"]),
        "wo2": np.ascontiguousarray(np.asarray(inputs["Wo2"], f)),
        "w1": np.ascontiguousarray(np.asarray(inputs["W1"], f).astype(_bf16)),
        "w2": np.ascontiguousarray(np.asarray(inputs["W2"], f).astype(_bf16)),
    }
    lng = np.stack([inputs["ln1_g"], inputs["ln2_g"], inputs["ln3_g"]]).astype(f)
    lnb = np.stack([inputs["ln1_b"], inputs["ln2_b"], inputs["ln3_b"]]).astype(f)
    bf1 = np.asarray(inputs["bf1"], f)
    bf2 = np.asarray(inputs["bf2"], f)
    trivial_ln = bool(np.all(lng == 1.0) and np.all(lnb == 0.0))
    trivial_bias = bool(np.all(bf1 == 0.0) and np.all(bf2 == 0.0))
    if not trivial_ln:
        shared["lng"] = np.ascontiguousarray(lng)
        shared["lnb"] = np.ascontiguousarray(lnb)
    if not trivial_bias:
        shared["bf1"] = np.ascontiguousarray(bf1)
        shared["bf2"] = np.ascontiguousarray(bf2)

    in_maps = []
    for c in range(NCORES):
        xc = x[c * BLOC:(c + 1) * BLOC].reshape(TLOC, D).T
        yc = y[c * BLOC:(c + 1) * BLOC].reshape(TLOC, D).T
        m = dict(shared)
        m["xT"] = np.ascontiguousarray(xc)
        m["yT"] = np.ascontiguousarray(yc)
        in_maps.append(m)
    return in_maps, trivial_ln, trivial_bias, x.shape


def run(inputs, trace=False, tmpdir=None):
    in_maps, trivial_ln, trivial_bias, xshape = _prep_inputs(inputs)
    nc = _get_nc(trivial_ln, trivial_bias)
    res = run_bass_kernel_spmd(
        nc, in_maps, list(range(NCORES)), trace=trace, tmpdir=tmpdir)
    B = xshape[0]
    out = np.empty((B, S, D), np.float32)
    for c in range(NCORES):
        out[c * BLOC:(c + 1) * BLOC] = (
            res.results[c]["outT"].T.reshape(BLOC, S, D))
    return out, res


def kernel(**inputs) -> np.ndarray:
    out, _ = run(inputs)
    return out



# revision 12
# speedup vs baseline: 1.0721x; 1.0721x over previous
"""Trainium2 Bass kernel for nn_Decoder (6-layer transformer decoder).

Strategy: data-parallel over batch B=16 across 8 NeuronCores (2 sequences
per core), weights replicated. Per core everything is computed feature-major
(activations stored transposed, [features on partitions, tokens on free dim])
so every linear layer is a weight-stationary matmul and no on-device
transposes are needed:

  - projections:   out^T = W.T @ x^T        (W is already [d_in, d_out])
  - scores:        S^T[t,q] = K_h Q_h^T     (K=64 contraction, heads packed
                                             two-per-PE-pass at partition 0/64)
  - softmax:       P = exp(S^T/8 + causal mask); denominator comes for free
                   from an all-ones column appended to token-major V
  - AV:            O^T = [V;1]^T P  -> [65, 512] PSUM, row 64 = denominator
  - layernorm:     token-wise stats via all-ones stationary matmuls
                   (replicated over partitions), rstd = exp(-0.5*ln(var+eps))
                   so ACT stays on the natural_log_exp table set

All matmul operands are bf16 (HW requires both inputs same width; bf16
gets fast-weight-load and full-rate small-N matmuls). The residual stream
and LN math stay fp32; every LN additionally emits a bf16 shadow copy that
feeds the next block's matmuls. y is loaded and cast to bf16 once.

Host side transposes x/y/weights into these layouts (numpy), shards the
batch, and runs the single compiled Bass program SPMD on cores 0-7.
"""
import sys

if "/opt/trn_rl_repo" not in sys.path:
    sys.path.insert(0, "/opt/trn_rl_repo")

import ml_dtypes
import numpy as np

import concourse.bass as bass
import concourse.mybir as mybir
import concourse.tile as tile
from concourse import bacc
from concourse.bass_utils import run_bass_kernel_spmd

# The ACT-table placement pass maps Exp -> "exp_and_others" and Ln ->
# "natural_log", so a kernel using both thrashes ACT_TABLE_LOADs (~1.3us
# each) inside the softmax/LN chain.  Advertise Exp/Ln only from the
# combined "natural_log_exp_and_others" set (indices are preserved, so the
# emitted act_func_set_id still matches act_info.json) -> one load total.
_orig_get_act_tables = bacc.get_activation_tables


def _patched_get_act_tables(arch):
    tables = dict(_orig_get_act_tables(arch))
    exp = mybir.ActivationFunctionType.Exp
    ln = mybir.ActivationFunctionType.Ln
    if any(exp in f and ln in f for f in tables.values()):
        out = {}
        for name, fns in tables.items():
            if exp in fns and ln in fns:
                out[name] = fns
            else:
                out[name] = fns - {exp, ln}
        return out
    return tables


bacc.get_activation_tables = _patched_get_act_tables

_bf16 = ml_dtypes.bfloat16
F32R = mybir.dt.float32r
F32 = mybir.dt.float32
BF16 = mybir.dt.bfloat16
AF = mybir.ActivationFunctionType
ALU = mybir.AluOpType

L, H, D, DH, DFF = 6, 8, 512, 64, 2048
EPS = 1e-5
NCORES = 8
BLOC = 2            # sequences per core
S = 512             # tokens per sequence
TLOC = BLOC * S     # tokens per core
KT = D // 128       # 4 contraction k-tiles for D
MT = D // 128       # 4 output feature m-tiles
NH = 2              # FFN token chunks (512 tokens each)
FCH = TLOC // NH    # 512


def _build(trivial_ln: bool, trivial_bias: bool):
    nc = bacc.Bacc("TRN2", target_bir_lowering=False, debug=False)

    xT = nc.dram_tensor("xT", [D, TLOC], F32R, kind="ExternalInput")
    yT = nc.dram_tensor("yT", [D, TLOC], F32R, kind="ExternalInput")
    wq1 = nc.dram_tensor("wq1", [L, D, D], BF16, kind="ExternalInput")
    wk1 = nc.dram_tensor("wk1", [L, D, D], BF16, kind="ExternalInput")
    wv1 = nc.dram_tensor("wv1", [L, D, D], BF16, kind="ExternalInput")
    wo1 = nc.dram_tensor("wo1", [L, D, D], BF16, kind="ExternalInput")
    wq2 = nc.dram_tensor("wq2", [L, D, D], BF16, kind="ExternalInput")
    wk2 = nc.dram_tensor("wk2", [L, D, D], BF16, kind="ExternalInput")
    wv2 = nc.dram_tensor("wv2", [L, D, D], BF16, kind="ExternalInput")
    wo2 = nc.dram_tensor("wo2", [L, D, D], BF16, kind="ExternalInput")
    w1 = nc.dram_tensor("w1", [L, D, DFF], BF16, kind="ExternalInput")
    w2 = nc.dram_tensor("w2", [L, DFF, D], BF16, kind="ExternalInput")
    if not trivial_ln:
        lng = nc.dram_tensor("lng", [3, L, D], F32, kind="ExternalInput")
        lnb = nc.dram_tensor("lnb", [3, L, D], F32, kind="ExternalInput")
    if not trivial_bias:
        bf1 = nc.dram_tensor("bf1", [L, DFF], F32, kind="ExternalInput")
        bf2 = nc.dram_tensor("bf2", [L, D], F32, kind="ExternalInput")
    outT = nc.dram_tensor("outT", [D, TLOC], F32R, kind="ExternalOutput")

    from contextlib import ExitStack
    with ExitStack() as _ctx:
        tc = _ctx.enter_context(tile.TileContext(nc))
        _ctx.enter_context(nc.allow_low_precision(reason="bf16 matmul inputs"))

        def _pool(name, bufs, space="SBUF"):
            return _ctx.enter_context(tc.tile_pool(name=name, bufs=bufs, space=space))

        cst = _pool("cst", 1)
        hidp = _pool("hid", 6)
        hbp = _pool("hb", 6)       # bf16 shadows of the residual stream
        sresp = _pool("sres", 4)
        sqp = _pool("sq", 1)
        qkp = _pool("qk", 4)
        vvp = _pool("vv", 4)
        ptp = _pool("pt", 3)
        denp = _pool("den", 1)
        recp = _pool("rec", 2)
        osbp = _pool("osb", 3)
        oalp = _pool("oal", 4)
        f1p = _pool("f1", 16)
        awqp = _pool("awq", 4)
        awkp = _pool("awk", 4)
        awvp = _pool("awv", 4)
        awop = _pool("awo", 4)
        w1p = _pool("w1p", 4)
        w2p = _pool("w2p", 16)
        ybp = _pool("ybp", 4)
        stp = _pool("st", 4)
        t1p = _pool("t1", 3)
        b1p = _pool("b1", 16)
        tinyp = _pool("tiny", 4)
        psA = _pool("psA", 4, "PSUM")
        psB = _pool("psB", 3, "PSUM")
        psC = _pool("psC", 1, "PSUM")

        if True:
            # ---------------- constants ----------------
            onesf = cst.tile([128, 128], F32, tag="onesf", name="onesf")
            nc.gpsimd.memset(onesf[:], 1.0)
            ones16 = cst.tile([128, 128], BF16, tag="ones16", name="ones16")
            nc.vector.tensor_copy(ones16[:], onesf[:])
            ones = cst.tile([128, 128], F32R, tag="ones", name="ones")
            nc.vector.tensor_copy(ones[:], onesf[:])
            # causal diag-block mask: keep (0) where t(p) <= q(j), else -1e30
            maskD = cst.tile([128, 128], F32, tag="maskD")
            nc.gpsimd.memset(maskD[:], 0.0)
            nc.gpsimd.affine_select(
                out=maskD[:], in_=maskD[:], compare_op=ALU.is_ge,
                fill=-1e30, base=0, pattern=[[1, 128]], channel_multiplier=-1,
            )
            epsb = cst.tile([128, 1], F32, tag="epsb", name="epsb")
            nc.gpsimd.memset(epsb[:], EPS)

            # ---------------- initial activation load ----------------
            xcur, xcur_b = [], []
            for k in range(KT):
                t = hidp.tile([128, TLOC], F32R, tag="hid", name="hid")
                nc.sync.dma_start(t[:], xT.ap()[k * 128:(k + 1) * 128, :])
                xcur.append(t)
                tb = hbp.tile([128, TLOC], BF16, tag="hb", name="hb")
                nc.vector.tensor_copy(tb[:], t[:])
                xcur_b.append(tb)
            # persistent bf16 copy of the encoder output y (K/V source for
            # every cross-attention)
            yb16 = []
            for k in range(KT):
                t = ybp.tile([128, TLOC], F32R, tag="yb", name="yb")
                nc.sync.dma_start(t[:], yT.ap()[k * 128:(k + 1) * 128, :])
                tb = ybp.tile([128, TLOC], BF16, tag="yb16", name="yb16")
                nc.vector.tensor_copy(tb[:], t[:])
                yb16.append(tb)

            # persistent token-major V tiles (4 t-tiles), with the all-ones
            # denominator column written once
            vper = []
            for i in range(4):
                va = vvp.tile([128, H * 65], BF16, tag="vv", name="vv")
                nc.vector.tensor_copy(
                    va[:].rearrange("p (h c) -> p h c", h=H)[:, :, 64:65],
                    onesf[:, 0:H].rearrange("p (h o) -> p h o", o=1))
                vper.append(va)

            def load_proj_w(pool, dram, l, tag):
                tiles = []
                for k in range(KT):
                    t = pool.tile([128, D], BF16, tag=tag)
                    nc.sync.dma_start(t[:], dram.ap()[l, k * 128:(k + 1) * 128, :])
                    tiles.append(t)
                return tiles

            def ln_params(idx, l):
                """per-feature gain/bias columns for LN idx (0..2) of layer l."""
                if trivial_ln:
                    return None, None
                gs, bs = [], []
                for k in range(KT):
                    g = tinyp.tile([128, 1], F32, tag="lng", name="lng")
                    nc.sync.dma_start(
                        g[:], lng.ap()[idx, l, k * 128:(k + 1) * 128].rearrange("p -> p 1"))
                    b = tinyp.tile([128, 1], F32, tag="lnb", name="lnb")
                    nc.sync.dma_start(
                        b[:], lnb.ap()[idx, l, k * 128:(k + 1) * 128].rearrange("p -> p 1"))
                    gs.append(g)
                    bs.append(b)
                return gs, bs

            def layer_norm(src, idx, l, last=False):
                """src: 4 tiles [128, TLOC] fp32r. Returns (4 fp32 hid tiles,
                4 bf16 shadow tiles)."""
                gs, bs = ln_params(idx, l)
                outb = [hbp.tile([128, TLOC], BF16, tag="hb", name="hb")
                        for _ in range(KT)]
                sq = []
                for k in range(KT):
                    s = sqp.tile([128, TLOC], BF16, tag="sq", name="sq")
                    nc.gpsimd.tensor_mul(s[:], src[k][:], src[k][:])
                    sq.append(s)
                out = [hidp.tile([128, TLOC], F32R, tag="hid", name="hid") for _ in range(KT)]
                for nh in range(2):
                    cs = slice(nh * 512, (nh + 1) * 512)
                    sum_ps = psB.tile([128, 512], F32, tag="sc", name="sc")
                    ssq_ps = psB.tile([128, 512], F32, tag="sc", name="sc")
                    for k in range(KT):
                        nc.tensor.matmul(sum_ps[:], ones[:], src[k][:, cs],
                                         start=(k == 0), stop=(k == KT - 1))
                    for k in range(KT):
                        nc.tensor.matmul(ssq_ps[:], ones16[:], sq[k][:, cs],
                                         start=(k == 0), stop=(k == KT - 1))
                    m = stp.tile([128, 512], F32, tag="st", name="st")
                    nc.vector.tensor_scalar_mul(m[:], sum_ps[:], 1.0 / D)
                    t1s = []
                    for k in range(KT):
                        t1 = t1p.tile([128, 512], F32, tag="t1", name="t1")
                        nc.vector.tensor_sub(t1[:], src[k][:, cs], m[:])
                        t1s.append(t1)
                    msq = stp.tile([128, 512], F32, tag="st", name="st")
                    nc.vector.tensor_mul(msq[:], m[:], m[:])
                    var = stp.tile([128, 512], F32, tag="st", name="st")
                    nc.vector.scalar_tensor_tensor(
                        var[:], ssq_ps[:], 1.0 / D, msq[:], ALU.mult, ALU.subtract)
                    lnv = stp.tile([128, 512], F32, tag="st", name="st")
                    nc.scalar.activation(lnv[:], var[:], AF.Ln, bias=epsb[:])
                    rstd = stp.tile([128, 512], F32, tag="st", name="st")
                    nc.scalar.activation(rstd[:], lnv[:], AF.Exp, scale=-0.5)
                    for k in range(KT):
                        if trivial_ln:
                            # bf16 shadow first: it feeds the next block's
                            # matmuls, so it must not queue behind the fp32
                            # write
                            nc.vector.tensor_mul(
                                outb[k][:, cs], t1s[k][:], rstd[:])
                            nc.vector.tensor_mul(out[k][:, cs], t1s[k][:], rstd[:])
                        else:
                            t2 = t1p.tile([128, 512], F32, tag="t2", name="t2")
                            nc.vector.tensor_mul(t2[:], t1s[k][:], rstd[:])
                            nc.vector.tensor_scalar(
                                outb[k][:, cs], t2[:], gs[k][:], bs[k][:],
                                ALU.mult, ALU.add)
                            nc.vector.tensor_scalar(
                                out[k][:, cs], t2[:], gs[k][:], bs[k][:],
                                ALU.mult, ALU.add)
                        if last:
                            nc.sync.dma_start(
                                outT.ap()[k * 128:(k + 1) * 128, cs], out[k][:, cs])
                return out, outb

            def attention(qsrc_b, kv_from_y, l, wq_d, wk_d, wv_d, wo_d, causal,
                          resid_src):
                """Full MHA block. qsrc_b: 4 bf16 fm tiles (queries). K/V from
                y (cross) or qsrc_b (self). Returns s = attn_out + resid
                (4 fp32 sres tiles)."""
                wq_t = load_proj_w(awqp, wq_d, l, "awq")
                wk_t = load_proj_w(awkp, wk_d, l, "awk")
                wv_t = load_proj_w(awvp, wv_d, l, "awv")
                wo_t = load_proj_w(awop, wo_d, l, "awo")
                oall = [oalp.tile([128, TLOC], BF16, tag="oal", name="oal") for _ in range(MT)]
                for b in range(BLOC):
                    bs = slice(b * S, (b + 1) * S)
                    kvs = yb16 if kv_from_y else qsrc_b
                    # Q/K projections, feature-major [D, S]
                    qb, kb = [], []
                    for (dst, wt, tg) in ((qb, wq_t, "q"), (kb, wk_t, "k")):
                        for mi in range(MT):
                            ps = psA.tile([128, 512], F32, tag="mm", name="mm")
                            for k in range(KT):
                                nc.tensor.matmul(
                                    ps[:], wt[k][:, mi * 128:(mi + 1) * 128],
                                    qsrc_b[k][:, bs] if tg == "q" else kvs[k][:, bs],
                                    start=(k == 0), stop=(k == KT - 1))
                            o = qkp.tile([128, S], BF16, tag=tg, name=tg)
                            if tg == "k":
                                nc.scalar.copy(o[:], ps[:])
                            else:
                                nc.vector.tensor_copy(o[:], ps[:])
                            dst.append(o)
                    # V token-major augmented: [128 tok, 8*(64+1)]
                    vb = []
                    for tt in range(4):
                        ps = psA.tile([128, 512], F32, tag="mm", name="mm")
                        for k in range(KT):
                            t0 = b * S + tt * 128
                            nc.tensor.matmul(
                                ps[:], kvs[k][:, t0:t0 + 128], wv_t[k][:],
                                start=(k == 0), stop=(k == KT - 1))
                        va = vper[tt]
                        nc.vector.tensor_copy(
                            va[:].rearrange("p (h c) -> p h c", h=H)[:, :, 0:64],
                            ps[:].rearrange("p (h c) -> p h c", h=H))
                        vb.append(va)
                    # heads, in pairs: softmax denominators for both heads
                    # land in one [33, 512] tile (rows 0 / 32) and get one
                    # batched reciprocal on the scalar engine as exp(-ln(x)).
                    # The normalize (bc matmul + mul) runs one group behind so
                    # the in-order PE stream never waits on the ACT chain.
                    pending = []

                    def flush_norm(item):
                        phg, prec, posbu = item
                        for hh in range(2):
                            h = phg * 2 + hh
                            hb = (h % 2) * 64
                            mi = h // 2
                            r = hh * 32
                            bc = psB.tile([64, 512], F32, tag="sc", name="sc")
                            nc.tensor.matmul(
                                bc[:], ones[r:r + 1, 0:64], prec[r:r + 1, :],
                                start=True, stop=True)
                            nc.vector.tensor_mul(
                                oall[mi][hb:hb + 64, bs], bc[:], posbu[hh][:])

                    for hg in range(H // 2):
                        if len(pending) > 1:
                            flush_norm(pending.pop(0))
                        den = denp.tile([33, 512], F32, tag="den", name="den")
                        osbu = []
                        for hh in range(2):
                            h = hg * 2 + hh
                            hb = (h % 2) * 64
                            mi = h // 2
                            pts = []
                            for tt in range(4):
                                n0 = tt * 128 if causal else 0
                                sc_ps = psB.tile([128, 512], F32, tag="sc", name="sc")
                                nc.tensor.matmul(
                                    sc_ps[:, n0:512],
                                    kb[mi][hb:hb + 64, tt * 128:(tt + 1) * 128],
                                    qb[mi][hb:hb + 64, n0:512],
                                    start=True, stop=True)
                                ptt = ptp.tile([128, 512], BF16, tag="pt", name="pt")
                                if causal:
                                    nc.vector.tensor_add(
                                        sc_ps[:, n0:n0 + 128], sc_ps[:, n0:n0 + 128],
                                        maskD[:])
                                nc.scalar.activation(
                                    ptt[:, n0:512], sc_ps[:, n0:512], AF.Exp,
                                    scale=0.125)
                                pts.append(ptt)
                            av = psC.tile([65, 512], F32, tag="av", name="av")
                            for tt in range(4):
                                n0 = tt * 128 if causal else 0
                                nc.tensor.matmul(
                                    av[:, n0:512],
                                    vb[tt][:, h * 65:h * 65 + 65],
                                    pts[tt][:, n0:512],
                                    start=(tt == 0), stop=(tt == 3))
                            nc.vector.tensor_copy(
                                den[hh * 32:hh * 32 + 1, :], av[64:65, :])
                            ou = osbp.tile([64, 512], F32, tag="osb", name="osb")
                            nc.vector.tensor_copy(ou[:], av[0:64, :])
                            osbu.append(ou)
                        lnd = denp.tile([33, 512], F32, tag="lnd", name="lnd")
                        nc.scalar.activation(lnd[:], den[:], AF.Ln)
                        rec = recp.tile([33, 512], F32R, tag="rec", name="rec")
                        nc.scalar.activation(rec[:], lnd[:], AF.Exp, scale=-1.0)
                        pending.append((hg, rec, osbu))
                    while pending:
                        flush_norm(pending.pop(0))
                # output projection + residual
                s_out = [sresp.tile([128, TLOC], F32R, tag="sres", name="sres") for _ in range(MT)]
                for mi in range(MT):
                    for nh in range(2):
                        cs = slice(nh * 512, (nh + 1) * 512)
                        ps = psA.tile([128, 512], F32, tag="mm", name="mm")
                        for k in range(KT):
                            nc.tensor.matmul(
                                ps[:], wo_t[k][:, mi * 128:(mi + 1) * 128],
                                oall[k][:, cs], start=(k == 0), stop=(k == KT - 1))
                        nc.vector.tensor_add(
                            s_out[mi][:, cs], ps[:], resid_src[mi][:, cs])
                return s_out

            # ================= layer loop =================
            for l in range(L):
                # FFN weights for this layer: issued up front on the (idle)
                # GPSIMD SWDGE queues so they never contend with the SP HWDGE
                # stream that feeds attention weights.
                w1_t = []
                for k in range(KT):
                    t = w1p.tile([128, DFF], BF16, tag="w1", name="w1")
                    nc.gpsimd.dma_start(
                        t[:], w1.ap()[l, k * 128:(k + 1) * 128, :])
                    w1_t.append(t)
                w2_t = []
                for k in range(DFF // 128):
                    t = w2p.tile([128, D], BF16, tag="w2", name="w2")
                    nc.gpsimd.dma_start(
                        t[:], w2.ap()[l, k * 128:(k + 1) * 128, :])
                    w2_t.append(t)
                # ---- masked self-attention + LN1 ----
                s1 = attention(xcur_b, False, l, wq1, wk1, wv1, wo1, True, xcur)
                h1, h1b = layer_norm(s1, 0, l)
                # ---- cross-attention + LN2 ----
                s2 = attention(h1b, True, l, wq2, wk2, wv2, wo2, False, h1)
                h2, h2b = layer_norm(s2, 1, l)
                # ---- FFN ----
                if not trivial_bias:
                    b1c, b2c = [], []
                    for i in range(DFF // 128):
                        t = b1p.tile([128, 1], F32, tag="b1c", name="b1c")
                        nc.sync.dma_start(
                            t[:], bf1.ap()[l, i * 128:(i + 1) * 128].rearrange("p -> p 1"))
                        b1c.append(t)
                    for i in range(MT):
                        t = tinyp.tile([128, 1], F32, tag="b2c", name="b2c")
                        nc.sync.dma_start(
                            t[:], bf2.ap()[l, i * 128:(i + 1) * 128].rearrange("p -> p 1"))
                        b2c.append(t)
                s3 = [sresp.tile([128, TLOC], F32R, tag="sres", name="sres") for _ in range(MT)]
                for nh in range(NH):
                    cs = slice(nh * FCH, (nh + 1) * FCH)
                    f1t = []
                    for m in range(DFF // 128):
                        ps = psA.tile([128, FCH], F32, tag="mm", name="mm")
                        for k in range(KT):
                            nc.tensor.matmul(
                                ps[:], w1_t[k][:, m * 128:(m + 1) * 128],
                                h2b[k][:, cs], start=(k == 0), stop=(k == KT - 1))
                        f = f1p.tile([128, FCH], BF16, tag="f1", name="f1")
                        # bias+relu on DVE (scalar engine is softmax-bound)
                        if trivial_bias:
                            nc.vector.tensor_scalar_max(f[:], ps[:], 0.0)
                        else:
                            nc.vector.tensor_scalar(
                                f[:], ps[:], b1c[m][:], 0.0, ALU.add, ALU.max)
                        f1t.append(f)
                    # FFN2: s3 = f1 @ W2 + bf2 + h2  (m outer so only one
                    # PSUM bank accumulates at a time)
                    for m in range(MT):
                        fps = psA.tile([128, FCH], F32, tag="mm", name="mm")
                        for k in range(DFF // 128):
                            nc.tensor.matmul(
                                fps[:], w2_t[k][:, m * 128:(m + 1) * 128],
                                f1t[k][:], start=(k == 0), stop=(k == DFF // 128 - 1))
                        nc.vector.scalar_tensor_tensor(
                            s3[m][:, cs], fps[:],
                            0.0 if trivial_bias else b2c[m][:],
                            h2[m][:, cs], ALU.add, ALU.add)
                xcur, xcur_b = layer_norm(s3, 2, l, last=(l == L - 1))

    nc.compile()
    return nc


_NC_CACHE = {}


def _get_nc(trivial_ln, trivial_bias):
    key = (trivial_ln, trivial_bias)
    if key not in _NC_CACHE:
        _NC_CACHE[key] = _build(trivial_ln, trivial_bias)
    return _NC_CACHE[key]


def _prep_inputs(inputs):
    f = np.float32
    x = np.asarray(inputs["x"], f)
    y = np.asarray(inputs["y"], f)

    def fm(w):  # [L, H, D, DH] -> [L, D, H*DH], bf16
        return np.ascontiguousarray(
            np.asarray(w, f).transpose(0, 2, 1, 3).reshape(L, D, H * DH)
            .astype(_bf16))

    shared = {
        "wq1": fm(inputs["Wq1"]), "wk1": fm(inputs["Wk1"]), "wv1": fm(inputs["Wv1"]),
        "wo1": np.ascontiguousarray(np.asarray(inputs["Wo1"], f).astype(_bf16)),
        "wq2": fm(inputs["Wq2"]), "wk2": fm(inputs["Wk2"]), "wv2": fm(inputs["Wv2"]),
        "wo2": np.ascontiguousarray(np.asarray(inputs["Wo2"], f).astype(_bf16)),
        "w1": np.ascontiguousarray(np.asarray(inputs["W1"], f).astype(_bf16)),
        "w2": np.ascontiguousarray(np.asarray(inputs["W2"], f).astype(_bf16)),
    }
    lng = np.stack([inputs["ln1_g"], inputs["ln2_g"], inputs["ln3_g"]]).astype(f)
    lnb = np.stack([inputs["ln1_b"], inputs["ln2_b"], inputs["ln3_b"]]).astype(f)
    bf1 = np.asarray(inputs["bf1"], f)
    bf2 = np.asarray(inputs["bf2"], f)
    trivial_ln = bool(np.all(lng == 1.0) and np.all(lnb == 0.0))
    trivial_bias = bool(np.all(bf1 == 0.0) and np.all(bf2 == 0.0))
    if not trivial_ln:
        shared["lng"] = np.ascontiguousarray(lng)
        shared["lnb"] = np.ascontiguousarray(lnb)
    if not trivial_bias:
        shared["bf1"] = np.ascontiguousarray(bf1)
        shared["bf2"] = np.ascontiguousarray(bf2)

    in_maps = []
    for c in range(NCORES):
        xc = x[c * BLOC:(c + 1) * BLOC].reshape(TLOC, D).T
        yc = y[c * BLOC:(c + 1) * BLOC].reshape(TLOC, D).T
        m = dict(shared)
        m["xT"] = np.ascontiguousarray(xc)
        m["yT"] = np.ascontiguousarray(yc)
        in_maps.append(m)
    return in_maps, trivial_ln, trivial_bias, x.shape


def run(inputs, trace=False, tmpdir=None):
    in_maps, trivial_ln, trivial_bias, xshape = _prep_inputs(inputs)
    nc = _get_nc(trivial_ln, trivial_bias)
    res = run_bass_kernel_spmd(
        nc, in_maps, list(range(NCORES)), trace=trace, tmpdir=tmpdir)
    B = xshape[0]
    out = np.empty((B, S, D), np.float32)
    for c in range(NCORES):
        out[c * BLOC:(c + 1) * BLOC] = (
            res.results[c]["outT"].T.reshape(BLOC, S, D))
    return out, res


def kernel(**inputs) -> np.ndarray:
    out, _ = run(inputs)
    return out


# revision 24
# speedup vs baseline: 1.2137x; 1.1320x over previous
"""Trainium2 Bass kernel for nn_Decoder (6-layer transformer decoder).

Strategy: data-parallel over batch B=16 across 8 NeuronCores (2 sequences
per core), weights replicated. Per core everything is computed feature-major
(activations stored transposed, [features on partitions, tokens on free dim])
so every linear layer is a weight-stationary matmul and no on-device
transposes are needed:

  - projections:   out^T = W.T @ x^T        (W is already [d_in, d_out])
  - scores:        S^T[t,q] = K_h Q_h^T     (K=64 contraction, heads packed
                                             two-per-PE-pass at partition 0/64)
  - softmax:       P = exp(S^T/8 + causal mask); denominator comes for free
                   from an all-ones column appended to token-major V
  - AV:            O^T = [V;1]^T P  -> [65, 512] PSUM, row 64 = denominator
  - layernorm:     token-wise stats via all-ones stationary matmuls
                   (replicated over partitions), rstd = exp(-0.5*ln(var+eps))
                   so ACT stays on the natural_log_exp table set

The two sequences are emitted as two interleaved instruction streams offset
by roughly one attention phase, so one stream's PE-heavy projections/FFN
fill the tensor-engine bubbles left by the other stream's ACT-bound softmax
and LN chains (the PE queue is in-order: interleaved emission is the only
way to keep it fed, which also keeps the HAM power state at full speed).

All matmul operands are bf16 (HW requires both inputs the same width; bf16
gets fast-weight-load and full-rate small-N matmuls). The residual stream
is kept in bf16; LN statistics and the normalize are computed in fp32 from
the fp32 pre-LN sums.

Host side transposes x/y/weights into these layouts (numpy), shards the
batch, and runs the single compiled Bass program SPMD on cores 0-7.
"""
import sys

if "/opt/trn_rl_repo" not in sys.path:
    sys.path.insert(0, "/opt/trn_rl_repo")

import ml_dtypes
import numpy as np

import concourse.bass as bass
import concourse.mybir as mybir
import concourse.tile as tile
from concourse import bacc
from concourse.bass_utils import run_bass_kernel_spmd

# The ACT-table placement pass maps Exp -> "exp_and_others" and Ln ->
# "natural_log", so a kernel using both thrashes ACT_TABLE_LOADs (~1.3us
# each) inside the softmax/LN chain.  Advertise Exp/Ln only from the
# combined "natural_log_exp_and_others" set (indices are preserved, so the
# emitted act_func_set_id still matches act_info.json) -> one load total.
_orig_get_act_tables = bacc.get_activation_tables


def _patched_get_act_tables(arch):
    tables = dict(_orig_get_act_tables(arch))
    exp = mybir.ActivationFunctionType.Exp
    ln = mybir.ActivationFunctionType.Ln
    if any(exp in f and ln in f for f in tables.values()):
        out = {}
        for name, fns in tables.items():
            if exp in fns and ln in fns:
                out[name] = fns
            else:
                out[name] = fns - {exp, ln}
        return out
    return tables


bacc.get_activation_tables = _patched_get_act_tables

_bf16 = ml_dtypes.bfloat16
F32R = mybir.dt.float32r
F32 = mybir.dt.float32
BF16 = mybir.dt.bfloat16
AF = mybir.ActivationFunctionType
ALU = mybir.AluOpType

L, H, D, DH, DFF = 6, 8, 512, 64, 2048
EPS = 1e-5
NCORES = 8
BLOC = 2            # sequences per core
S = 512             # tokens per sequence
TLOC = BLOC * S     # tokens per core
KT = D // 128       # 4 contraction k-tiles for D
MT = D // 128       # 4 output feature m-tiles
FT = DFF // 128     # 16 FFN hidden m-tiles

# stream-1 lags stream-0 by about one attention phase (in driver yields)
PIPE_OFFSET = 54


def _build(trivial_ln: bool, trivial_bias: bool):
    nc = bacc.Bacc("TRN2", target_bir_lowering=False, debug=False)

    xT = nc.dram_tensor("xT", [D, TLOC], F32R, kind="ExternalInput")
    yT = nc.dram_tensor("yT", [D, TLOC], F32R, kind="ExternalInput")
    wq1 = nc.dram_tensor("wq1", [L, D, D], BF16, kind="ExternalInput")
    wk1 = nc.dram_tensor("wk1", [L, D, D], BF16, kind="ExternalInput")
    wv1 = nc.dram_tensor("wv1", [L, D, D], BF16, kind="ExternalInput")
    wo1 = nc.dram_tensor("wo1", [L, D, D], BF16, kind="ExternalInput")
    wq2 = nc.dram_tensor("wq2", [L, D, D], BF16, kind="ExternalInput")
    wk2 = nc.dram_tensor("wk2", [L, D, D], BF16, kind="ExternalInput")
    wv2 = nc.dram_tensor("wv2", [L, D, D], BF16, kind="ExternalInput")
    wo2 = nc.dram_tensor("wo2", [L, D, D], BF16, kind="ExternalInput")
    w1 = nc.dram_tensor("w1", [L, D, DFF], BF16, kind="ExternalInput")
    w2 = nc.dram_tensor("w2", [L, DFF, D], BF16, kind="ExternalInput")
    if not trivial_ln:
        lng = nc.dram_tensor("lng", [3, L, D], F32, kind="ExternalInput")
        lnb = nc.dram_tensor("lnb", [3, L, D], F32, kind="ExternalInput")
    if not trivial_bias:
        bf1 = nc.dram_tensor("bf1", [L, DFF], F32, kind="ExternalInput")
        bf2 = nc.dram_tensor("bf2", [L, D], F32, kind="ExternalInput")
    outT = nc.dram_tensor("outT", [D, TLOC], F32R, kind="ExternalOutput")

    from contextlib import ExitStack
    with ExitStack() as _ctx:
        tc = _ctx.enter_context(tile.TileContext(nc))
        _ctx.enter_context(nc.allow_low_precision(reason="bf16 matmul inputs"))

        def _pool(name, bufs, space="SBUF"):
            return _ctx.enter_context(tc.tile_pool(name=name, bufs=bufs, space=space))

        cst = _pool("cst", 1)
        ybpool = _pool("ybp", 4)
        stp = _pool("st", 4)
        f1p = _pool("f1", 16)
        b1p = _pool("b1", 16)
        tinyp = _pool("tiny", 4)
        # per-attention-type weight pools (one layer resident each)
        aw1p = [_pool(f"aw1_{i}", 4) for i in range(4)]   # self q/k/v/o
        aw2p = [_pool(f"aw2_{i}", 4) for i in range(4)]   # cross q/k/v/o
        w1p = _pool("w1p", 4)
        w2p = _pool("w2p", 16)
        # per-stream pools
        hbp = [_pool(f"hb{b}", 8) for b in range(2)]
        sresp = [_pool(f"sres{b}", 5) for b in range(2)]
        sqp = [_pool(f"sq{b}", 4) for b in range(2)]
        qkp = [_pool(f"qk{b}", 4) for b in range(2)]
        vvp = [_pool(f"vv{b}", 4) for b in range(2)]
        ptp = [_pool(f"pt{b}", 5) for b in range(2)]
        denp = [_pool(f"den{b}", 1) for b in range(2)]
        recp = [_pool(f"rec{b}", 2) for b in range(2)]
        osbp = [_pool(f"osb{b}", 2) for b in range(2)]
        oalp = [_pool(f"oal{b}", 4) for b in range(2)]
        psA = _pool("psA", 2, "PSUM")
        psS = [_pool(f"psS{b}", 2, "PSUM") for b in range(2)]
        psAV = [_pool(f"psAV{b}", 1, "PSUM") for b in range(2)]

        # ---------------- constants ----------------
        onesf = cst.tile([128, 128], F32, tag="onesf", name="onesf")
        nc.gpsimd.memset(onesf[:], 1.0)
        ones16 = cst.tile([128, 128], BF16, tag="ones16", name="ones16")
        nc.vector.tensor_copy(ones16[:], onesf[:])
        ones = cst.tile([128, 128], F32R, tag="ones", name="ones")
        nc.vector.tensor_copy(ones[:], onesf[:])
        # causal diag-block mask: keep (0) where t(p) <= q(j), else -1e30
        maskD = cst.tile([128, 128], F32, tag="maskD")
        nc.gpsimd.memset(maskD[:], 0.0)
        nc.gpsimd.affine_select(
            out=maskD[:], in_=maskD[:], compare_op=ALU.is_ge,
            fill=-1e30, base=0, pattern=[[1, 128]], channel_multiplier=-1,
        )
        epsb = cst.tile([128, 1], F32, tag="epsb", name="epsb")
        nc.gpsimd.memset(epsb[:], EPS)

        # ---------------- initial activation load ----------------
        # x: per-stream bf16 residual tiles; y: shared bf16 K/V source
        xcb0 = [[], []]
        for b in range(2):
            for k in range(KT):
                stg = sresp[b].tile([128, S], F32R, tag="sres", name="xstg")
                nc.sync.dma_start(
                    stg[:], xT.ap()[k * 128:(k + 1) * 128, b * S:(b + 1) * S])
                tb = hbp[b].tile([128, S], BF16, tag="hb", name="hb")
                nc.vector.tensor_copy(tb[:], stg[:])
                xcb0[b].append(tb)
        yb16 = []
        for k in range(KT):
            tb = ybpool.tile([128, TLOC], BF16, tag="yb16", name="yb16")
            for b in range(2):
                stg = sresp[b].tile([128, S], F32R, tag="sres", name="ystg")
                nc.sync.dma_start(
                    stg[:], yT.ap()[k * 128:(k + 1) * 128, b * S:(b + 1) * S])
                nc.vector.tensor_copy(tb[:, b * S:(b + 1) * S], stg[:])
            yb16.append(tb)

        # persistent token-major V tiles per stream, with the all-ones
        # denominator column written once
        vper = [[], []]
        for b in range(2):
            for i in range(4):
                va = vvp[b].tile([128, H * 65], BF16, tag="vv", name="vv")
                nc.vector.tensor_copy(
                    va[:].rearrange("p (h c) -> p h c", h=H)[:, :, 64:65],
                    onesf[:, 0:H].rearrange("p (h o) -> p h o", o=1))
                vper[b].append(va)

        # ---------------- weight caches (leader stream loads) ----------
        wcache = {}

        def get_attn_w(l, which):
            key = (l, which)
            if key not in wcache:
                pools = aw1p if which == 1 else aw2p
                drams = ((wq1, wk1, wv1, wo1) if which == 1
                         else (wq2, wk2, wv2, wo2))
                sets = []
                for pool, dram in zip(pools, drams):
                    tiles = []
                    for k in range(KT):
                        t = pool.tile([128, D], BF16, tag="w", name="w")
                        nc.sync.dma_start(
                            t[:], dram.ap()[l, k * 128:(k + 1) * 128, :])
                        tiles.append(t)
                    sets.append(tiles)
                wcache[key] = tuple(sets)
            return wcache[key]

        def get_ffn_w(l):
            key = (l, "ffn")
            if key not in wcache:
                w1_t = []
                for k in range(KT):
                    t = w1p.tile([128, DFF], BF16, tag="w1", name="w1")
                    nc.sync.dma_start(
                        t[:], w1.ap()[l, k * 128:(k + 1) * 128, :])
                    w1_t.append(t)
                w2_t = []
                for k in range(FT):
                    t = w2p.tile([128, D], BF16, tag="w2", name="w2")
                    nc.sync.dma_start(
                        t[:], w2.ap()[l, k * 128:(k + 1) * 128, :])
                    w2_t.append(t)
                wcache[key] = (w1_t, w2_t)
            return wcache[key]

        def get_bias(l):
            key = (l, "bias")
            if key not in wcache:
                b1c, b2c = [], []
                for i in range(FT):
                    t = b1p.tile([128, 1], F32, tag="b1c", name="b1c")
                    nc.sync.dma_start(
                        t[:], bf1.ap()[l, i * 128:(i + 1) * 128].rearrange("p -> p 1"))
                    b1c.append(t)
                for i in range(MT):
                    t = tinyp.tile([128, 1], F32, tag="b2c", name="b2c")
                    nc.sync.dma_start(
                        t[:], bf2.ap()[l, i * 128:(i + 1) * 128].rearrange("p -> p 1"))
                    b2c.append(t)
                wcache[key] = (b1c, b2c)
            return wcache[key]

        def get_ln_params(idx, l):
            if trivial_ln:
                return None, None
            key = (l, "ln", idx)
            if key not in wcache:
                gs, bs = [], []
                for k in range(KT):
                    g = tinyp.tile([128, 1], F32, tag="lng", name="lng")
                    nc.sync.dma_start(
                        g[:], lng.ap()[idx, l, k * 128:(k + 1) * 128].rearrange("p -> p 1"))
                    b = tinyp.tile([128, 1], F32, tag="lnb", name="lnb")
                    nc.sync.dma_start(
                        b[:], lnb.ap()[idx, l, k * 128:(k + 1) * 128].rearrange("p -> p 1"))
                    gs.append(g)
                    bs.append(b)
                wcache[key] = (gs, bs)
            return wcache[key]

        # ---------------- per-stream generators ----------------
        def attn(b, qs, kvt, koff, wset, causal, resid):
            """qs/resid: 4 bf16 [128,S] tiles. kvt: tiles sliced at koff.
            Returns 4 fp32 s tiles (attn_out + resid)."""
            wq_t, wk_t, wv_t, wo_t = wset
            # Q / K projections, feature-major [D, S]
            qb, kb = [], []
            for (dst, wt, src, soff, tg) in (
                    (qb, wq_t, qs, 0, "q"), (kb, wk_t, kvt, koff, "k")):
                for mi in range(MT):
                    ps = psA.tile([128, 512], F32, tag="mm", name="mm")
                    for k in range(KT):
                        nc.tensor.matmul(
                            ps[:], wt[k][:, mi * 128:(mi + 1) * 128],
                            src[k][:, soff:soff + S],
                            start=(k == 0), stop=(k == KT - 1))
                    o = qkp[b].tile([128, S], BF16, tag=tg, name=tg)
                    if tg == "k":
                        nc.scalar.copy(o[:], ps[:])
                    else:
                        nc.vector.tensor_copy(o[:], ps[:])
                    dst.append(o)
                    yield
            # V token-major augmented: [128 tok, 8*(64+1)]
            for tt in range(4):
                ps = psA.tile([128, 512], F32, tag="mm", name="mm")
                t0 = koff + tt * 128
                for k in range(KT):
                    nc.tensor.matmul(
                        ps[:], kvt[k][:, t0:t0 + 128], wv_t[k][:],
                        start=(k == 0), stop=(k == KT - 1))
                va = vper[b][tt]
                nc.vector.tensor_copy(
                    va[:].rearrange("p (h c) -> p h c", h=H)[:, :, 0:64],
                    ps[:].rearrange("p (h c) -> p h c", h=H))
                yield
            # attention core, head pairs.  AV goes through a single PSUM
            # bank per stream; GPSIMD immediately stages it (plus the
            # denominator row) to SBUF so the bank frees for the next head.
            # The reciprocal+broadcast+normalize for pair g runs behind
            # pair g+1's scores so the PE never waits on the ACT chain.
            oall = [oalp[b].tile([128, S], BF16, tag="oal", name="oal")
                    for _ in range(MT)]
            pending = []

            def flush_norm(item):
                pg, prec, posb = item
                bc = psS[b].tile([128, 512], F32, tag="sc", name="sc")
                for hh in range(2):
                    nc.tensor.matmul(
                        bc[hh * 64:hh * 64 + 64, :],
                        ones16[hh * 32:hh * 32 + 1, 0:64],
                        prec[hh * 32:hh * 32 + 1, :], start=True, stop=True)
                nc.vector.tensor_mul(oall[pg][:], bc[:], posb[:])

            for g in range(4):
                osb = osbp[b].tile([128, S], F32, tag="osb", name="osb")
                den = denp[b].tile([33, S], F32, tag="den", name="den")
                for hh in range(2):
                    h = 2 * g + hh
                    hb_ = hh * 64
                    pts = []
                    for tt in range(4):
                        n0 = tt * 128 if causal else 0
                        sc = psS[b].tile([128, 512], F32, tag="sc", name="sc")
                        nc.tensor.matmul(
                            sc[:, n0:512],
                            kb[g][hb_:hb_ + 64, tt * 128:(tt + 1) * 128],
                            qb[g][hb_:hb_ + 64, n0:512],
                            start=True, stop=True)
                        pt = ptp[b].tile([128, 512], BF16, tag="pt", name="pt")
                        if causal:
                            nc.vector.tensor_add(
                                sc[:, n0:n0 + 128], sc[:, n0:n0 + 128], maskD[:])
                        nc.scalar.activation(
                            pt[:, n0:512], sc[:, n0:512], AF.Exp, scale=0.125)
                        pts.append(pt)
                        if tt == 1:
                            yield
                    if hh == 1 and pending:
                        flush_norm(pending.pop(0))
                    yield
                    av = psAV[b].tile([65, 512], F32, tag="av", name="av")
                    for tt in range(4):
                        n0 = tt * 128 if causal else 0
                        nc.tensor.matmul(
                            av[:, n0:512],
                            vper[b][tt][:, h * 65:h * 65 + 65],
                            pts[tt][:, n0:512],
                            start=(tt == 0), stop=(tt == 3))
                    yield
                    nc.vector.tensor_copy(osb[hb_:hb_ + 64, :], av[0:64, :])
                    nc.vector.tensor_copy(
                        den[hh * 32:hh * 32 + 1, :], av[64:65, :])
                    yield
                nc.scalar.activation(den[:], den[:], AF.Ln)
                rec = recp[b].tile([33, S], BF16, tag="rec", name="rec")
                nc.scalar.activation(rec[:], den[:], AF.Exp, scale=-1.0)
                pending.append((g, rec, osb))
            # output projection + residual (flush last pair first)
            s_out = []
            for mi in range(MT):
                if pending:
                    flush_norm(pending.pop(0))
                ps = psA.tile([128, 512], F32, tag="mm", name="mm")
                for k in range(KT):
                    nc.tensor.matmul(
                        ps[:], wo_t[k][:, mi * 128:(mi + 1) * 128],
                        oall[k][:], start=(k == 0), stop=(k == KT - 1))
                so = sresp[b].tile([128, S], F32R, tag="sres", name="sres")
                nc.vector.tensor_add(so[:], ps[:], resid[mi][:])
                s_out.append(so)
                yield
            return s_out

        def layer_norm(b, src, idx, l, last=False):
            """src: 4 fp32 [128,S] tiles. Returns 4 bf16 shadow tiles (the
            residual stream); for the last LN also DMAs fp32 out."""
            gs, bs_ = get_ln_params(idx, l)
            sq = []
            for k in range(KT):
                q = sqp[b].tile([128, S], BF16, tag="sq", name="sq")
                nc.gpsimd.tensor_mul(q[:], src[k][:], src[k][:])
                sq.append(q)
            yield
            sum_ps = psA.tile([128, 512], F32, tag="mm", name="mm")
            for k in range(KT):
                nc.tensor.matmul(sum_ps[:], ones[:], src[k][:],
                                 start=(k == 0), stop=(k == KT - 1))
            yield
            ssq_ps = psS[b].tile([128, 512], F32, tag="sc", name="sc")
            for k in range(KT):
                nc.tensor.matmul(ssq_ps[:], ones16[:], sq[k][:],
                                 start=(k == 0), stop=(k == KT - 1))
            yield
            m = stp.tile([128, 512], F32, tag="st", name="st")
            nc.vector.tensor_scalar_mul(m[:], sum_ps[:], 1.0 / D)
            # work tile morphs in place: msq -> var -> ln(var+eps) -> rstd
            rstd = stp.tile([128, 512], F32, tag="st", name="st")
            nc.vector.tensor_mul(rstd[:], m[:], m[:])
            nc.vector.scalar_tensor_tensor(
                rstd[:], ssq_ps[:], 1.0 / D, rstd[:], ALU.mult, ALU.subtract)
            nc.scalar.activation(rstd[:], rstd[:], AF.Ln, bias=epsb[:])
            nc.scalar.activation(rstd[:], rstd[:], AF.Exp, scale=-0.5)
            yield
            outb = []
            for k in range(KT):
                # src is spent after the stats: subtract the mean in place
                nc.vector.tensor_sub(src[k][:], src[k][:], m[:])
                ob = hbp[b].tile([128, S], BF16, tag="hb", name="hb")
                if trivial_ln:
                    nc.vector.tensor_mul(ob[:], src[k][:], rstd[:])
                    if last:
                        of = sresp[b].tile([128, S], F32R, tag="sres", name="of")
                        nc.vector.tensor_mul(of[:], src[k][:], rstd[:])
                        nc.sync.dma_start(
                            outT.ap()[k * 128:(k + 1) * 128, b * S:(b + 1) * S],
                            of[:])
                else:
                    nc.vector.tensor_mul(src[k][:], src[k][:], rstd[:])
                    nc.vector.tensor_scalar(
                        ob[:], src[k][:], gs[k][:], bs_[k][:], ALU.mult, ALU.add)
                    if last:
                        of = sresp[b].tile([128, S], F32R, tag="sres", name="of")
                        nc.vector.tensor_scalar(
                            of[:], src[k][:], gs[k][:], bs_[k][:], ALU.mult, ALU.add)
                        nc.sync.dma_start(
                            outT.ap()[k * 128:(k + 1) * 128, b * S:(b + 1) * S],
                            of[:])
                outb.append(ob)
                if k % 2 == 1:
                    yield
            return outb

        def ffn(b, h2b, l):
            """h2b: 4 bf16 tiles (LN2 out = FFN input and residual).
            Returns 4 fp32 s tiles."""
            w1_t, w2_t = get_ffn_w(l)
            if not trivial_bias:
                b1c, b2c = get_bias(l)
            f1t = []
            for m in range(FT):
                ps = psA.tile([128, 512], F32, tag="mm", name="mm")
                for k in range(KT):
                    nc.tensor.matmul(
                        ps[:], w1_t[k][:, m * 128:(m + 1) * 128],
                        h2b[k][:], start=(k == 0), stop=(k == KT - 1))
                f = f1p.tile([128, S], BF16, tag="f1", name="f1")
                # bias+relu on DVE (scalar engine is softmax-bound)
                if trivial_bias:
                    nc.vector.tensor_scalar_max(f[:], ps[:], 0.0)
                else:
                    nc.vector.tensor_scalar(
                        f[:], ps[:], b1c[m][:], 0.0, ALU.add, ALU.max)
                f1t.append(f)
                yield
            s3 = []
            for m in range(MT):
                # no yield between the psA alloc and its STT consumer: the
                # shared-ring slot must free without depending on PE work
                # emitted after another stream's allocation of it
                ps = psA.tile([128, 512], F32, tag="mm", name="mm")
                for k in range(FT):
                    nc.tensor.matmul(
                        ps[:], w2_t[k][:, m * 128:(m + 1) * 128],
                        f1t[k][:], start=(k == 0), stop=(k == FT - 1))
                sm = sresp[b].tile([128, S], F32R, tag="sres", name="sres")
                nc.vector.scalar_tensor_tensor(
                    sm[:], ps[:], 0.0 if trivial_bias else b2c[m][:],
                    h2b[m][:], ALU.add, ALU.add)
                s3.append(sm)
                yield
            return s3

        def stream(b):
            xcb = xcb0[b]
            for l in range(L):
                wset1 = get_attn_w(l, 1)
                s1 = yield from attn(b, xcb, xcb, 0, wset1, True, xcb)
                h1b = yield from layer_norm(b, s1, 0, l)
                wset2 = get_attn_w(l, 2)
                get_ffn_w(l)
                if not trivial_bias:
                    get_bias(l)
                s2 = yield from attn(b, h1b, yb16, b * S, wset2, False, h1b)
                h2b = yield from layer_norm(b, s2, 1, l)
                s3 = yield from ffn(b, h2b, l)
                xcb = yield from layer_norm(b, s3, 2, l, last=(l == L - 1))

        # ---------------- interleaved driver ----------------
        gens = [stream(0), stream(1)]
        done = [False, False]
        for _ in range(PIPE_OFFSET):
            try:
                next(gens[0])
            except StopIteration:
                done[0] = True
                break
        while not all(done):
            for i in range(2):
                if not done[i]:
                    try:
                        next(gens[i])
                    except StopIteration:
                        done[i] = True

    nc.compile()
    return nc


_NC_CACHE = {}


def _get_nc(trivial_ln, trivial_bias):
    key = (trivial_ln, trivial_bias)
    if key not in _NC_CACHE:
        _NC_CACHE[key] = _build(trivial_ln, trivial_bias)
    return _NC_CACHE[key]


def _prep_inputs(inputs):
    f = np.float32
    x = np.asarray(inputs["x"], f)
    y = np.asarray(inputs["y"], f)

    def fm(w):  # [L, H, D, DH] -> [L, D, H*DH], bf16
        return np.ascontiguousarray(
            np.asarray(w, f).transpose(0, 2, 1, 3).reshape(L, D, H * DH)
            .astype(_bf16))

    shared = {
        "wq1": fm(inputs["Wq1"]), "wk1": fm(inputs["Wk1"]), "wv1": fm(inputs["Wv1"]),
        "wo1": np.ascontiguousarray(np.asarray(inputs["Wo1"], f).astype(_bf16)),
        "wq2": fm(inputs["Wq2"]), "wk2": fm(inputs["Wk2"]), "wv2": fm(inputs["Wv2"]),
        "wo2": np.ascontiguousarray(np.asarray(inputs["Wo2"], f).astype(_bf16)),
        "w1": np.ascontiguousarray(np.asarray(inputs["W1"], f).astype(_bf16)),
        "w2": np.ascontiguousarray(np.asarray(inputs["W2"], f).astype(_bf16)),
    }
    lng = np.stack([inputs["ln1_g"], inputs["ln2_g"], inputs["ln3_g"]]).astype(f)
    lnb = np.stack([inputs["ln1_b"], inputs["ln2_b"], inputs["ln3_b"]]).astype(f)
    bf1 = np.asarray(inputs["bf1"], f)
    bf2 = np.asarray(inputs["bf2"], f)
    trivial_ln = bool(np.all(lng == 1.0) and np.all(lnb == 0.0))
    trivial_bias = bool(np.all(bf1 == 0.0) and np.all(bf2 == 0.0))
    if not trivial_ln:
        shared["lng"] = np.ascontiguousarray(lng)
        shared["lnb"] = np.ascontiguousarray(lnb)
    if not trivial_bias:
        shared["bf1"] = np.ascontiguousarray(bf1)
        shared["bf2"] = np.ascontiguousarray(bf2)

    in_maps = []
    for c in range(NCORES):
        xc = x[c * BLOC:(c + 1) * BLOC].reshape(TLOC, D).T
        yc = y[c * BLOC:(c + 1) * BLOC].reshape(TLOC, D).T
        m = dict(shared)
        m["xT"] = np.ascontiguousarray(xc)
        m["yT"] = np.ascontiguousarray(yc)
        in_maps.append(m)
    return in_maps, trivial_ln, trivial_bias, x.shape


def run(inputs, trace=False, tmpdir=None):
    in_maps, trivial_ln, trivial_bias, xshape = _prep_inputs(inputs)
    nc = _get_nc(trivial_ln, trivial_bias)
    res = run_bass_kernel_spmd(
        nc, in_maps, list(range(NCORES)), trace=trace, tmpdir=tmpdir)
    B = xshape[0]
    out = np.empty((B, S, D), np.float32)
    for c in range(NCORES):
        out[c * BLOC:(c + 1) * BLOC] = (
            res.results[c]["outT"].T.reshape(BLOC, S, D))
    return out, res


def kernel(**inputs) -> np.ndarray:
    out, _ = run(inputs)
    return out
